# revision 1
# baseline (speedup 1.0000x reference)
"""Causal self-attention (RMSNorm-QK + RoPE) Trainium2 Bass kernel.

Problem: B=2, T=2048, C=1024, H=16 heads, D=64.
Sharding: 8 cores = 2 (batch) x 4 (head groups of 4 heads).
Each core computes q/k/v projections for its 4 heads, attention, and a
partial output projection (column-parallel over heads); the host sums the
4 partials per batch and transposes.

All matmuls run in float32r (TF32-like, ~13-bit mantissa, 4x fp32 matmul
speed). f32r matmul operands must be produced by rounding ops or f32r DMA;
host pre-rounds the DRAM inputs.

Per-core layouts ("T-layout" = channels on partitions, tokens free):
  projection chunks [128, 512]: row 32h+i = head h, rope-half dim i
  qT_r/kT_r  2 x [128, 2048] f32r : chunk c rows 64*(h%2)+d = head 2c+h%2
  v_r        16 x [128, 260] f32r : head h at cols 65h..65h+63, ones col
  scoresT    [s-chunk 128, t-block 512]; softmax denom = ones-column row
  yT_sb      2 x [128, 2048] f32r : pair chunk c = heads (2c, 2c+1)
Output: outT [1024, 2048] = (partial out).T per core; host sums + transposes.
"""

import sys

for _p in ("/opt/trn_rl_repo",):
    if _p not in sys.path:
        sys.path.append(_p)

import numpy as np

B, T, C = 2, 2048, 1024
H_TOT, D = 16, 64
HPC = 4               # heads per core
N_CORES = 8
P = 128               # partitions
NB = 4                # t-blocks of 512
TB = 512              # t-block size
KCH = 8               # C / 128 contraction chunks
VW = 65 * HPC         # v width with ones columns = 260
RMS_EPS = 1.1920928955078125e-07
ROPE_BASE = 10000.0

_CACHE = {}


def _build_consts():
    """Host-side constant tensors shared by all cores."""
    inv_freq = (1.0 / (ROPE_BASE ** (np.arange(0, D, 2, dtype=np.float32) / np.float32(D)))).astype(np.float32)
    pos = np.arange(T, dtype=np.float32)
    freqs = np.outer(pos, inv_freq).astype(np.float32)      # [T, 32]
    cos = np.cos(freqs).astype(np.float32)                  # [T, 32]
    sin = np.sin(freqs).astype(np.float32)
    cosr = np.ascontiguousarray(np.tile(cos.T, (HPC, 1)))   # [128, T]
    sinr = np.ascontiguousarray(np.tile(sin.T, (HPC, 1)))
    # ind32 [128, 4]: per-32-row-group summing matrix (lhsT for RMS sums)
    ind32 = np.zeros((P, HPC), dtype=np.float32)
    for p_ in range(P):
        ind32[p_, p_ // 32] = 1.0
    # bc32 [4, 128]: broadcast inv (4 heads) to 32-row groups (lhsT)
    bc32 = np.zeros((HPC, P), dtype=np.float32)
    for p_ in range(P):
        bc32[p_ // 32, p_] = 1.0
    # selpair [128, 256]: chunk c (=0,1): col m -> den row 32*(2c + m//64)
    selpair = np.zeros((P, 2 * P), dtype=np.float32)
    for c in range(2):
        for m in range(P):
            selpair[32 * (2 * c + m // 64), 128 * c + m] = 1.0
    return dict(cosr=cosr, sinr=sinr, ind32=ind32, bc32=bc32,
                selpair=selpair)


def _build_module():
    import concourse.bacc as bacc
    import concourse.mybir as mybir
    import concourse.tile as tile

    f32 = mybir.dt.float32
    f32r = mybir.dt.float32r
    Exp = mybir.ActivationFunctionType.Exp
    Ln = mybir.ActivationFunctionType.Ln
    Alu = mybir.AluOpType

    nc = bacc.Bacc("TRN2", target_bir_lowering=False, debug=False,
                   num_devices=N_CORES)

    xt_d = nc.dram_tensor("xt", [C, T], f32r, kind="ExternalInput").ap()
    wq_d = nc.dram_tensor("wq", [C, 256], f32r, kind="ExternalInput").ap()
    wk_d = nc.dram_tensor("wk", [C, 256], f32r, kind="ExternalInput").ap()
    wv_d = nc.dram_tensor("wv", [C, VW], f32r, kind="ExternalInput").ap()
    wp_d = nc.dram_tensor("wp", [256, C], f32r, kind="ExternalInput").ap()
    cosr_d = nc.dram_tensor("cosr", [P, T], f32, kind="ExternalInput").ap()
    sinr_d = nc.dram_tensor("sinr", [P, T], f32, kind="ExternalInput").ap()
    ind32_d = nc.dram_tensor("ind32", [P, HPC], f32r, kind="ExternalInput").ap()
    bc32_d = nc.dram_tensor("bc32", [HPC, P], f32r, kind="ExternalInput").ap()
    selpair_d = nc.dram_tensor("selpair", [P, 2 * P], f32r, kind="ExternalInput").ap()
    zeros_d = nc.dram_tensor("zeros", [64, T], f32r, kind="ExternalInput").ap()
    out_d = nc.dram_tensor("outT", [C, T], f32, kind="ExternalOutput").ap()

    with tile.TileContext(nc) as tc:
        with (
            tc.tile_pool(name="sb", bufs=1) as sb,
            tc.tile_pool(name="trans", bufs=2) as tr,
            tc.tile_pool(name="ps", bufs=2, space="PSUM") as ps,
        ):
            # ---- constants / weights in (direct f32r DMA) ----
            def direct_load(name, dram_slice, shape, dt=f32r):
                t_r = sb.tile(shape, dt, tag=name, name=name)
                nc.sync.dma_start(out=t_r[:], in_=dram_slice)
                return t_r

            ind32_r = direct_load("ind32r", ind32_d[:, :], [P, HPC])
            bc32_r = direct_load("bc32r", bc32_d[:, :], [HPC, P])
            selpair_r = direct_load("selpairr", selpair_d[:, :], [P, 2 * P])
            cosr_t = direct_load("cosr", cosr_d[:, :], [P, T], f32)
            sinr_t = direct_load("sinr", sinr_d[:, :], [P, T], f32)
            wq_r = [direct_load(f"wqr{k}", wq_d[k * P:(k + 1) * P, :], [P, 256])
                    for k in range(KCH)]
            wk_r = [direct_load(f"wkr{k}", wk_d[k * P:(k + 1) * P, :], [P, 256])
                    for k in range(KCH)]
            wv_r = [direct_load(f"wvr{k}", wv_d[k * P:(k + 1) * P, :], [P, VW])
                    for k in range(KCH)]
            wp_r = [direct_load(f"wpr{c}", wp_d[c * P:(c + 1) * P, :], [P, C])
                    for c in range(2)]

            # ---- persistent intermediates ----
            qT_r = [sb.tile([P, T], f32r, tag=f"qT{c}", name=f"qT{c}")
                    for c in range(2)]
            # kTe[c]: rows 0-63 = head 2c, rows 64-127 zero;
            # kTo[c]: rows 0-63 zero, rows 64-127 = head 2c+1.
            kTe = [sb.tile([P, T], f32r, tag=f"kTe{c}", name=f"kTe{c}")
                   for c in range(2)]
            kTo = [sb.tile([P, T], f32r, tag=f"kTo{c}", name=f"kTo{c}")
                   for c in range(2)]
            for c in range(2):
                nc.sync.dma_start(out=kTe[c][64:128, :], in_=zeros_d[:, :])
                nc.sync.dma_start(out=kTo[c][0:64, :], in_=zeros_d[:, :])
            v_r = [sb.tile([P, VW], f32r, tag=f"v{s}", name=f"v{s}")
                   for s in range(T // P)]
            yT_sb = [sb.tile([P, T], f32r, tag=f"yT{c}", name=f"yT{c}")
                     for c in range(2)]
            den_stack = sb.tile([P, T], f32r, tag="denstack", name="den_stack")
            nc.gpsimd.memset(den_stack[:].bitcast(f32), 1.0)
            eps_t = sb.tile([HPC, 1], f32, tag="epst", name="eps_t")
            nc.gpsimd.memset(eps_t[:], RMS_EPS)

            # ====== Phase 1+2: projections, RMS-norm, RoPE, repack ======
            for n in range(NB):
                nsl = slice(n * TB, (n + 1) * TB)
                xr_t = []
                for k in range(KCH):
                    xr = tr.tile([P, TB], f32r, tag="xr", name=f"xr{n}_{k}", bufs=9)
                    nc.sync.dma_start(out=xr[:], in_=xt_d[k * P:(k + 1) * P, nsl])
                    xr_t.append(xr)
                pq0 = ps.tile([P, TB], f32, tag="psA", name=f"pq0_{n}")
                pq1 = ps.tile([P, TB], f32, tag="psA", name=f"pq1_{n}")
                pk0 = ps.tile([P, TB], f32, tag="psB", name=f"pk0_{n}")
                pk1 = ps.tile([P, TB], f32, tag="psB", name=f"pk1_{n}")
                for k in range(KCH):
                    xr = xr_t[k]
                    st = (k == 0)
                    sp = (k == KCH - 1)
                    nc.tensor.matmul(pq0[:], lhsT=wq_r[k][:, 0:128], rhs=xr[:],
                                     start=st, stop=sp)
                    nc.tensor.matmul(pq1[:], lhsT=wq_r[k][:, 128:256], rhs=xr[:],
                                     start=st, stop=sp)
                    nc.tensor.matmul(pk0[:], lhsT=wk_r[k][:, 0:128], rhs=xr[:],
                                     start=st, stop=sp)
                    nc.tensor.matmul(pk1[:], lhsT=wk_r[k][:, 128:256], rhs=xr[:],
                                     start=st, stop=sp)
                # q/k chunks out of PSUM
                x1q = tr.tile([P, TB], f32, tag="x1q", name=f"x1q{n}", bufs=1)
                x2q = tr.tile([P, TB], f32, tag="x2q", name=f"x2q{n}", bufs=1)
                x1k = tr.tile([P, TB], f32, tag="x1k", name=f"x1k{n}", bufs=1)
                x2k = tr.tile([P, TB], f32, tag="x2k", name=f"x2k{n}", bufs=1)
                nc.vector.tensor_copy(x1q[:], pq0[:])
                nc.vector.tensor_copy(x2q[:], pq1[:])
                nc.vector.tensor_copy(x1k[:], pk0[:])
                nc.vector.tensor_copy(x2k[:], pk1[:])
                # v projections (second sub-pass over the same xr tiles)
                pv = [ps.tile([P, VW], f32, tag=("psA" if s < 2 else "psB"),
                              name=f"pv{n}_{s}") for s in range(4)]
                for k in range(KCH):
                    st = (k == 0)
                    sp = (k == KCH - 1)
                    for s_rel in range(4):
                        nc.tensor.matmul(
                            pv[s_rel][:],
                            lhsT=xr_t[k][:, s_rel * P:(s_rel + 1) * P],
                            rhs=wv_r[k][:], start=st, stop=sp)
                for s_rel in range(4):
                    vt = v_r[4 * n + s_rel]
                    nc.vector.tensor_copy(vt[:], pv[s_rel][:])
                    nc.vector.tensor_scalar(vt[:, 64:VW:65], pv[s_rel][:, 64:VW:65],
                                            0.0, 1.0, Alu.mult, Alu.add)
                # RMS-norm + RoPE + repack, per tensor
                for (x1, x2, dstT, eng) in ((x1q, x2q, qT_r, "q"),
                                            (x1k, x2k, None, "k")):
                    e = nc.vector if eng == "q" else nc.gpsimd
                    sq1 = tr.tile([P, TB], f32r, tag="tmpA", name=f"sq1{eng}{n}", bufs=1)
                    sq2 = tr.tile([P, TB], f32r, tag="tmpB", name=f"sq2{eng}{n}", bufs=1)
                    nc.gpsimd.tensor_mul(sq1[:], x1[:], x1[:])
                    nc.gpsimd.tensor_mul(sq2[:], x2[:], x2[:])
                    ps_s = ps.tile([HPC, TB], f32, tag="psA", name=f"pss{eng}{n}")
                    nc.tensor.matmul(ps_s[:], lhsT=ind32_r[:], rhs=sq1[:],
                                     start=True, stop=False)
                    nc.tensor.matmul(ps_s[:], lhsT=ind32_r[:], rhs=sq2[:],
                                     start=False, stop=True)
                    invc = tr.tile([HPC, TB], f32r, tag="invc", name=f"invc{eng}{n}")
                    nc.scalar.activation(invc[:], ps_s[:], Ln,
                                         bias=eps_t[:], scale=1.0 / 64.0)
                    nc.scalar.activation(invc[:], invc[:], Exp, scale=-0.5)
                    ps_b = ps.tile([P, TB], f32, tag="psB", name=f"psb{eng}{n}")
                    nc.tensor.matmul(ps_b[:], lhsT=bc32_r[:], rhs=invc[:],
                                     start=True, stop=True)
                    nc.vector.tensor_mul(x1[:], x1[:], ps_b[:])
                    nc.vector.tensor_mul(x2[:], x2[:], ps_b[:])
                    # rope
                    m_a = tr.tile([P, TB], f32, tag="tmpA", name=f"ma{eng}{n}", bufs=1)
                    m_b = tr.tile([P, TB], f32, tag="tmpB", name=f"mb{eng}{n}", bufs=1)
                    rc1 = tr.tile([P, TB], f32r, tag="roch1", name=f"rc1{eng}{n}", bufs=1)
                    rc2 = tr.tile([P, TB], f32r, tag="roch2", name=f"rc2{eng}{n}", bufs=1)
                    e.tensor_mul(m_a[:], x1[:], cosr_t[:, nsl])
                    e.tensor_mul(m_b[:], x2[:], sinr_t[:, nsl])
                    e.tensor_add(rc1[:], m_a[:], m_b[:])
                    m_c = tr.tile([P, TB], f32, tag="tmpA", name=f"mc{eng}{n}", bufs=1)
                    m_d = tr.tile([P, TB], f32, tag="tmpB", name=f"md{eng}{n}", bufs=1)
                    e.tensor_mul(m_c[:], x2[:], cosr_t[:, nsl])
                    e.tensor_mul(m_d[:], x1[:], sinr_t[:, nsl])
                    e.tensor_sub(rc2[:], m_c[:], m_d[:])
                    # repack: head h rows 32h..32h+32 of (rc1|rc2) ->
                    # q: qT_r[h//2] rows 64*(h%2)..; k: kTe/kTo (zero-padded)
                    for h in range(HPC):
                        if eng == "q":
                            dst = dstT[h // 2]
                            rb = 64 * (h % 2)
                        else:
                            dst = (kTe if h % 2 == 0 else kTo)[h // 2]
                            rb = 64 * (h % 2)
                        hs = slice(32 * h, 32 * h + 32)
                        nc.sync.dma_start(out=dst[rb:rb + 32, nsl], in_=rc1[hs, :])
                        nc.sync.dma_start(out=dst[rb + 32:rb + 64, nsl], in_=rc2[hs, :])

            # ================= Phase 3: attention =================
            for h in range(HPC):
                cch = h // 2
                kT_h = (kTe if h % 2 == 0 else kTo)[cch]
                rsl = slice(64 * (h % 2), 64 * (h % 2) + 64)
                pa = "psC"
                ya = "psD"
                et_tag = "expT" if h % 2 == 0 else "expT2"
                for j in range(NB):
                    jsl = slice(j * TB, (j + 1) * TB)
                    n_k = 4 * j + 4
                    Yh = ps.tile([65, TB], f32, tag=ya, name=f"Y{h}_{j}")
                    for k in range(n_k):
                        ksl = slice(k * P, (k + 1) * P)
                        st, sp = (k == 0), (k == n_k - 1)
                        r = k - 4 * j          # >=0 on diagonal blocks
                        # cols t < 128r of this block are fully masked; trim
                        # matmuls to N>=256 (f32r full-rate) and exp always.
                        mtrim = 128 * r if 0 < r <= 2 else 0
                        etrim = 128 * r if r > 0 else 0
                        msl = slice(mtrim, TB)
                        esl = slice(etrim, TB)
                        S0 = ps.tile([P, TB], f32, tag=pa, name=f"S{h}_{j}_{k}")
                        nc.tensor.matmul(S0[:, msl], lhsT=kT_h[:, ksl],
                                         rhs=qT_r[cch][:, j * TB + mtrim:(j + 1) * TB],
                                         start=True, stop=True)
                        e0 = tr.tile([P, TB], f32r, tag=et_tag,
                                     name=f"e{h}_{j}_{k}", bufs=3)
                        nc.scalar.activation(e0[:, esl], S0[:, esl], Exp, scale=0.125)
                        if r >= 0:  # diagonal: apply causal mask (zero-fills left)
                            e0m = tr.tile([P, TB], f32r, tag=et_tag,
                                          name=f"em{h}_{j}_{k}", bufs=3)
                            nc.gpsimd.affine_select(
                                out=e0m[:], in_=e0[:], pattern=[[1, TB]],
                                compare_op=Alu.is_ge, fill=0.0,
                                base=-128 * r, channel_multiplier=-1)
                            e0 = e0m
                        nc.tensor.matmul(Yh[:, msl], lhsT=v_r[k][:, 65 * h:65 * h + 65],
                                         rhs=e0[:, msl], start=st, stop=sp)
                    # copy out: y rows + den row (SBUF bounce; DMA shifts rows)
                    yb = tr.tile([65, TB], f32r, tag="cpbuf", name=f"yb{h}_{j}",
                                 bufs=3, padded_shape=[P, TB])
                    nc.vector.tensor_copy(yb[:], Yh[:])
                    nc.sync.dma_start(out=yT_sb[cch][rsl, jsl], in_=yb[0:64, :])
                    nc.sync.dma_start(out=den_stack[32 * h:32 * h + 1, jsl],
                                      in_=yb[64:65, :])

            # ================= Phase 4: normalize + out-projection ======
            # invden = exp(-ln(den)) on rows 0,32,64,96 (others memset to 1)
            invden_r = sb.tile([P, T], f32r, tag="invden", name="invden_r")
            nc.scalar.activation(den_stack[:], den_stack[:], Ln)
            nc.scalar.activation(invden_r[:], den_stack[:], Exp, scale=-1.0)
            for c in range(2):
                for n in range(NB):
                    nsl = slice(n * TB, (n + 1) * TB)
                    ps_i = ps.tile([P, TB], f32, tag="psA", name=f"psi{c}{n}")
                    nc.tensor.matmul(ps_i[:], lhsT=selpair_r[:, c * P:(c + 1) * P],
                                     rhs=invden_r[:, nsl], start=True, stop=True)
                    nc.vector.tensor_mul(yT_sb[c][:, nsl], yT_sb[c][:, nsl], ps_i[:])
            for o in range(8):
                osl = slice(o * P, (o + 1) * P)
                for n in range(NB):
                    nsl = slice(n * TB, (n + 1) * TB)
                    po = ps.tile([P, TB], f32, tag="psB", name=f"po{o}_{n}")
                    nc.tensor.matmul(po[:], lhsT=wp_r[0][:, osl], rhs=yT_sb[0][:, nsl],
                                     start=True, stop=False)
                    nc.tensor.matmul(po[:], lhsT=wp_r[1][:, osl], rhs=yT_sb[1][:, nsl],
                                     start=False, stop=True)
                    ob = tr.tile([P, TB], f32, tag="cpbuf", name=f"ob{o}_{n}", bufs=3)
                    nc.vector.tensor_copy(ob[:], po[:])
                    nc.sync.dma_start(out=out_d[osl, nsl], in_=ob[:])

    nc.compile()
    return nc


def _get_module():
    if "nc" not in _CACHE:
        _CACHE["nc"] = _build_module()
        _CACHE["consts"] = _build_consts()
    return _CACHE["nc"], _CACHE["consts"]


def _round_f32r(a, bits=10):
    u = np.ascontiguousarray(a, dtype=np.float32).view(np.uint32).astype(np.uint64)
    u = (u + (1 << (bits - 1))) & ~np.uint64((1 << bits) - 1)
    return np.minimum(u, 0xFFFFFFFF).astype(np.uint32).view(np.float32)


def _core_inputs(x, w_q, w_k, w_v, w_proj, core):
    """Build the per-core input map (numpy, host-side sharding)."""
    b = core // 4
    g = core % 4
    heads = [4 * g + j for j in range(HPC)]

    xt = _round_f32r(np.ascontiguousarray(x[b].T))        # [C, T]

    perm = np.empty(256, dtype=np.int64)
    for m in range(128):
        perm[m] = 64 * heads[m // 32] + (m % 32)             # x1 half
        perm[128 + m] = 64 * heads[m // 32] + 32 + (m % 32)  # x2 half
    wq = _round_f32r(np.ascontiguousarray(w_q[perm, :].T))   # [C, 256]
    wk = _round_f32r(np.ascontiguousarray(w_k[perm, :].T))

    # v weights with zero columns at 65h+64 (device writes the ones there)
    wv_aug = np.zeros((C, VW), dtype=np.float32)
    for j in range(HPC):
        wv_aug[:, 65 * j:65 * j + 64] = w_v[64 * heads[j]:64 * heads[j] + 64, :].T
    wv = _round_f32r(wv_aug)

    vperm = np.empty(256, dtype=np.int64)
    for m in range(256):
        vperm[m] = 64 * heads[m // 64] + (m % 64)
    wp = _round_f32r(np.ascontiguousarray(w_proj[:, vperm].T))  # [256, C]

    zeros = np.zeros((64, T), dtype=np.float32)
    return dict(xt=xt, wq=wq, wk=wk, wv=wv, wp=wp, zeros=zeros)


def kernel(x, w_q, w_k, w_v, w_proj, _trace=False, _trace_cores=None):
    from concourse.bass_utils import run_bass_kernel_spmd

    nc, consts = _get_module()
    x = np.asarray(x, dtype=np.float32)
    in_maps = []
    for core in range(N_CORES):
        m = _core_inputs(np.asarray(x), np.asarray(w_q), np.asarray(w_k),
                         np.asarray(w_v), np.asarray(w_proj), core)
        m.update(consts)
        in_maps.append(m)

    res = run_bass_kernel_spmd(nc, in_maps, list(range(N_CORES)),
                               trace=_trace, trace_cores=_trace_cores)
    outs = [res.results[c]["outT"] for c in range(N_CORES)]
    out = np.empty((B, T, C), dtype=np.float32)
    for b in range(B):
        acc = outs[4 * b].astype(np.float32)
        for g in range(1, 4):
            acc = acc + outs[4 * b + g]
        out[b] = acc.T
    if _trace:
        kernel._last_exec_time_ns = res.exec_time_ns
        kernel._last_results = res
    return out



# revision 41
# speedup vs baseline: 1.2478x; 1.2478x over previous
"""Causal self-attention (RMSNorm-QK + RoPE) Trainium2 Bass kernel.

Problem: B=2, T=2048, C=1024, H=16 heads, D=64.
Sharding: 8 cores = 2 (batch) x 4 (head groups of 4 heads).
Each core computes q/k/v projections for its 4 heads, attention, and a
partial output projection (column-parallel over heads); the host sums the
4 partials per batch and transposes.

All matmuls run in bf16 (inputs rounded on host) with f32 PSUM accumulation.
bf16 halves HBM traffic vs f32r and draws less PE power (avoids the HAM
half-clock throttle windows f32r provoked), and runs full-rate at any N.

Per-core layouts:
  projection chunks [128, 512]: row 32h+i = head h, rope-half dim i
  q1/q2/k1/k2[n]  [128, 512] bf16 : rope outputs per t-block, kept in the
      32h+i row layout; scores contract rc1+rc2 with two K=32 matmuls per
      head at PE row-group 32h (all 4 heads run concurrently).
  v_r[s]          [128, 4, 65] bf16 : key-chunk s, head h at [:, h, 0:64],
      ones column at [:, h, 64] (softmax denominator trick)
  S_A/S_B         [128, 2, 512] f32 PSUM : scores for heads (0,1) / (2,3)
  yT32[c]         [128, 2048] f32 : heads (2c, 2c+1) attention numerator
  yT_bf[c]        [128, 2048] bf16 : normalized (divided by denominator)
Output: outT [1024, 2048] bf16 = (partial out).T per core; host sums.
"""

import sys

for _p in ("/opt/trn_rl_repo",):
    if _p not in sys.path:
        sys.path.append(_p)

import numpy as np
import ml_dtypes

B, T, C = 2, 2048, 1024
H_TOT, D = 16, 64
HPC = 4               # heads per core
N_CORES = 8
P = 128               # partitions
NB = 4                # t-blocks of 512
TB = 512              # t-block size
KCH = 8               # C / 128 contraction chunks
RMS_EPS = 1.1920928955078125e-07
ROPE_BASE = 10000.0

_CACHE = {}


def _build_consts():
    """Host-side constant tensors shared by all cores."""
    inv_freq = (1.0 / (ROPE_BASE ** (np.arange(0, D, 2, dtype=np.float32) / np.float32(D)))).astype(np.float32)
    pos = np.arange(T, dtype=np.float32)
    freqs = np.outer(pos, inv_freq).astype(np.float32)      # [T, 32]
    cos = np.cos(freqs).astype(np.float32)                  # [T, 32]
    sin = np.sin(freqs).astype(np.float32)
    cosr = np.ascontiguousarray(np.tile(cos.T, (HPC, 1)))   # [128, T]
    sinr = np.ascontiguousarray(np.tile(sin.T, (HPC, 1)))
    # ind32 [128, 4]: per-32-row-group summing matrix (lhsT for RMS sums)
    ind32 = np.zeros((P, HPC), dtype=np.float32)
    for p_ in range(P):
        ind32[p_, p_ // 32] = 1.0
    # bc32 [4, 128]: broadcast inv (4 heads) to 32-row groups (lhsT)
    bc32 = np.zeros((HPC, P), dtype=np.float32)
    for p_ in range(P):
        bc32[p_ // 32, p_] = 1.0
    # selpair4 [4, 256]: pair c: out row m <- den row (2c + m//64)
    selpair4 = np.zeros((HPC, 2 * P), dtype=np.float32)
    for c in range(2):
        for m in range(P):
            selpair4[2 * c + m // 64, 128 * c + m] = 1.0
    # causal triangle mask [128, 2, 128] bf16 (same triangle both halves):
    # keep element (p, :, i) iff i >= p
    tri = (np.arange(P)[None, :] >= np.arange(P)[:, None]).astype(np.float32)
    maskt = np.ascontiguousarray(
        np.broadcast_to(tri[:, None, :], (P, 2, P))).astype(ml_dtypes.bfloat16)
    return dict(cosr=cosr, sinr=sinr, ind32=ind32.astype(ml_dtypes.bfloat16),
                bc32=bc32.astype(ml_dtypes.bfloat16),
                selpair4=selpair4.astype(ml_dtypes.bfloat16), maskt=maskt)


def _build_module():
    import concourse.bacc as bacc
    import concourse.mybir as mybir
    import concourse.tile as tile

    f32 = mybir.dt.float32
    f32r = mybir.dt.float32r
    bf16 = mybir.dt.bfloat16
    i32 = mybir.dt.int32
    Exp = mybir.ActivationFunctionType.Exp
    Ln = mybir.ActivationFunctionType.Ln
    Alu = mybir.AluOpType

    nc = bacc.Bacc("TRN2", target_bir_lowering=False, debug=False,
                   num_devices=N_CORES)

    xt_d = nc.dram_tensor("xt", [C, T], bf16, kind="ExternalInput").ap()
    wq_d = nc.dram_tensor("wq", [C, 256], bf16, kind="ExternalInput").ap()
    wk_d = nc.dram_tensor("wk", [C, 256], bf16, kind="ExternalInput").ap()
    wv_d = nc.dram_tensor("wv", [C, 256], bf16, kind="ExternalInput").ap()
    wp_d = nc.dram_tensor("wp", [256, C], bf16, kind="ExternalInput").ap()
    cosr_d = nc.dram_tensor("cosr", [P, T], f32, kind="ExternalInput").ap()
    sinr_d = nc.dram_tensor("sinr", [P, T], f32, kind="ExternalInput").ap()
    ind32_d = nc.dram_tensor("ind32", [P, HPC], bf16, kind="ExternalInput").ap()
    bc32_d = nc.dram_tensor("bc32", [HPC, P], bf16, kind="ExternalInput").ap()
    selpair4_d = nc.dram_tensor("selpair4", [HPC, 2 * P], bf16, kind="ExternalInput").ap()
    maskt_d = nc.dram_tensor("maskt", [P, 2, P], bf16, kind="ExternalInput").ap()
    out_d = nc.dram_tensor("outT", [C, T], bf16, kind="ExternalOutput").ap()
    DEBUG = bool(_CACHE.get("debug"))
    if DEBUG:
        dbg_q1 = nc.dram_tensor("dbg_q1", [P, TB], bf16, kind="ExternalOutput").ap()
        dbg_k1 = nc.dram_tensor("dbg_k1", [P, TB], bf16, kind="ExternalOutput").ap()
        dbg_y = nc.dram_tensor("dbg_y", [P, T], f32, kind="ExternalOutput").ap()
        dbg_den = nc.dram_tensor("dbg_den", [HPC, T], f32, kind="ExternalOutput").ap()
        dbg_inv = nc.dram_tensor("dbg_inv", [HPC, T], f32, kind="ExternalOutput").ap()
        dbg_v = nc.dram_tensor("dbg_v", [P, HPC, 65], bf16, kind="ExternalOutput").ap()
        dbg_x1q = nc.dram_tensor("dbg_x1q", [P, TB], f32, kind="ExternalOutput").ap()
        dbg_pv = nc.dram_tensor("dbg_pv", [P, 256], f32, kind="ExternalOutput").ap()
        dbg_wq = nc.dram_tensor("dbg_wq", [P, 256], bf16, kind="ExternalOutput").ap()
        dbg_xr = nc.dram_tensor("dbg_xr", [P, TB], bf16, kind="ExternalOutput").ap()

    with tile.TileContext(nc) as tc:
        with (
            tc.tile_pool(name="sb", bufs=1) as sb,
            tc.tile_pool(name="trans", bufs=2) as tr,
            tc.tile_pool(name="ps", bufs=1, space="PSUM") as ps,
        ):
            def direct_load(name, dram_slice, shape, dt):
                t_r = sb.tile(shape, dt, tag=name, name=name)
                nc.sync.dma_start(out=t_r[:], in_=dram_slice)
                return t_r

            # ---- small consts first ----
            ind32_r = direct_load("ind32r", ind32_d[:, :], [P, HPC], bf16)
            bc32_r = direct_load("bc32r", bc32_d[:, :], [HPC, P], bf16)
            selpair_r = direct_load("selpairr", selpair4_d[:, :], [HPC, 2 * P], bf16)
            mask_r = direct_load("maskr", maskt_d[:, :, :], [P, 2, P], bf16)
            wq_r = [direct_load(f"wqr{k}", wq_d[k * P:(k + 1) * P, :], [P, 256], bf16)
                    for k in range(KCH)]
            wk_r = [direct_load(f"wkr{k}", wk_d[k * P:(k + 1) * P, :], [P, 256], bf16)
                    for k in range(KCH)]

            eps_t = sb.tile([HPC, 1], f32, tag="epst", name="eps_t")
            nc.gpsimd.memset(eps_t[:], RMS_EPS)

            LOADS_TOP = bool(_CACHE.get("loads_top"))
            if LOADS_TOP:
                wv_r = [direct_load(f"wvr{k}", wv_d[k * P:(k + 1) * P, :], [P, 256], bf16)
                        for k in range(KCH)]
                cosr_t = direct_load("cosr", cosr_d[:, :], [P, T], f32)
                sinr_t = direct_load("sinr", sinr_d[:, :], [P, T], f32)
                wp_r = [direct_load(f"wpr{c}", wp_d[c * P:(c + 1) * P, :], [P, C], bf16)
                        for c in range(2)]

            # ---- persistent intermediates ----
            q1 = [sb.tile([P, TB], bf16, tag=f"q1_{n}", name=f"q1_{n}")
                  for n in range(NB)]
            q2 = [sb.tile([P, TB], bf16, tag=f"q2_{n}", name=f"q2_{n}")
                  for n in range(NB)]
            k1 = [sb.tile([P, TB], bf16, tag=f"k1_{n}", name=f"k1_{n}")
                  for n in range(NB)]
            k2 = [sb.tile([P, TB], bf16, tag=f"k2_{n}", name=f"k2_{n}")
                  for n in range(NB)]
            v_r = [sb.tile([P, HPC, 65], bf16, tag=f"v{s}", name=f"v{s}")
                   for s in range(T // P)]
            NO_VONES = bool(_CACHE.get("no_vones"))
            if not NO_VONES:
                for s in range(T // P):
                    nc.gpsimd.memset(v_r[s][:, :, 64:65], 1.0)
            yT32 = [sb.tile([P, T], f32, tag=f"yT32_{c}", name=f"yT32_{c}")
                    for c in range(2)]
            yT_bf = [sb.tile([P, T], bf16, tag=f"yTbf_{c}", name=f"yTbf_{c}")
                     for c in range(2)]
            den4 = sb.tile([HPC, T], f32, tag="den4", name="den4")
            invden4 = sb.tile([HPC, T], f32, tag="invden4", name="invden4")
            invden_bf = sb.tile([HPC, T], bf16, tag="invdenbf", name="invden_bf")

            PT = ["pA", "pB", "pC", "pD"]

            # ====== Phase 1: projections, RMS-norm, RoPE ======
            for n in range(NB):
                nsl = slice(n * TB, (n + 1) * TB)
                xr_t = []
                for k in range(KCH):
                    xr = tr.tile([P, TB], bf16, tag="xr", name=f"xr{n}_{k}", bufs=16)
                    nc.sync.dma_start(out=xr[:], in_=xt_d[k * P:(k + 1) * P, nsl])
                    xr_t.append(xr)
                if n == 0 and not LOADS_TOP:
                    # inputs not needed for the first q/k matmuls: load after
                    # block-0 x so compute starts as early as possible
                    wv_r = [direct_load(f"wvr{k}", wv_d[k * P:(k + 1) * P, :], [P, 256], bf16)
                            for k in range(KCH)]
                    cosr_t = direct_load("cosr", cosr_d[:, :], [P, T], f32)
                    sinr_t = direct_load("sinr", sinr_d[:, :], [P, T], f32)
                    wp_r = [direct_load(f"wpr{c}", wp_d[c * P:(c + 1) * P, :], [P, C], bf16)
                            for c in range(2)]
                pq0 = ps.tile([P, TB], f32, tag="pA", name=f"pq0_{n}")
                pq1 = ps.tile([P, TB], f32, tag="pB", name=f"pq1_{n}")
                pk0 = ps.tile([P, TB], f32, tag="pC", name=f"pk0_{n}")
                pk1 = ps.tile([P, TB], f32, tag="pD", name=f"pk1_{n}")
                for k in range(KCH):
                    xr = xr_t[k]
                    st = (k == 0)
                    sp = (k == KCH - 1)
                    nc.tensor.matmul(pq0[:], lhsT=wq_r[k][:, 0:128], rhs=xr[:],
                                     start=st, stop=sp)
                    nc.tensor.matmul(pq1[:], lhsT=wq_r[k][:, 128:256], rhs=xr[:],
                                     start=st, stop=sp)
                    nc.tensor.matmul(pk0[:], lhsT=wk_r[k][:, 0:128], rhs=xr[:],
                                     start=st, stop=sp)
                    nc.tensor.matmul(pk1[:], lhsT=wk_r[k][:, 128:256], rhs=xr[:],
                                     start=st, stop=sp)
                x1q = tr.tile([P, TB], f32, tag="x1q", name=f"x1q{n}", bufs=1)
                x2q = tr.tile([P, TB], f32, tag="x2q", name=f"x2q{n}", bufs=1)
                x1k = tr.tile([P, TB], f32, tag="x1k", name=f"x1k{n}", bufs=1)
                x2k = tr.tile([P, TB], f32, tag="x2k", name=f"x2k{n}", bufs=1)
                nc.vector.tensor_copy(x1q[:], pq0[:])
                nc.vector.tensor_copy(x2q[:], pq1[:])
                nc.vector.tensor_copy(x1k[:], pk0[:])
                nc.vector.tensor_copy(x2k[:], pk1[:])
                # v projections (token-major: lhsT = x chunk)
                NO_V = bool(_CACHE.get("no_v"))
                pv = [ps.tile([P, 256], f32, tag=PT[s], name=f"pv{n}_{s}")
                      for s in range(4)] if not NO_V else []
                for k in range(KCH) if not NO_V else ():
                    st = (k == 0)
                    sp = (k == KCH - 1)
                    for s_rel in range(4):
                        nc.tensor.matmul(
                            pv[s_rel][:],
                            lhsT=xr_t[k][:, s_rel * P:(s_rel + 1) * P],
                            rhs=wv_r[k][:], start=st, stop=sp)
                if DEBUG and n == 0:
                    if not NO_V:
                        dbg_pv_t = tr.tile([P, 256], f32, tag="dbgpv", name="dbg_pv_t", bufs=1)
                        nc.vector.tensor_copy(dbg_pv_t[:], pv[0][:])
                        nc.sync.dma_start(out=dbg_pv[:, :], in_=dbg_pv_t[:])
                    nc.sync.dma_start(out=dbg_x1q[:, :], in_=x1q[:])
                    nc.sync.dma_start(out=dbg_wq[:, :], in_=wq_r[0][:])
                    nc.sync.dma_start(out=dbg_xr[:, :], in_=xr_t[0][:])
                for s_rel in range(4) if not NO_V else ():
                    nc.vector.tensor_copy(v_r[4 * n + s_rel][:, :, 0:64], pv[s_rel][:])
                # RMS-norm + RoPE, per tensor (q on vector, k on gpsimd)
                NO_RMSROPE = bool(_CACHE.get("no_rmsrope"))
                for (x1, x2, d1, d2, eng) in (() if NO_RMSROPE else
                                              ((x1q, x2q, q1, q2, "q"),
                                               (x1k, x2k, k1, k2, "k"))):
                    e = nc.vector if eng == "q" else nc.gpsimd
                    ta, tb = (("tmpA", "tmpB") if eng == "q" else ("tmpC", "tmpD"))
                    sq1 = tr.tile([P, TB], bf16, tag=ta, name=f"sq1{eng}{n}", bufs=1)
                    sq2 = tr.tile([P, TB], bf16, tag=tb, name=f"sq2{eng}{n}", bufs=1)
                    e.tensor_mul(sq1[:], x1[:], x1[:])
                    e.tensor_mul(sq2[:], x2[:], x2[:])
                    ps_s = ps.tile([HPC, TB], f32, tag=("pA" if eng == "q" else "pB"),
                                   name=f"pss{eng}{n}")
                    nc.tensor.matmul(ps_s[:], lhsT=ind32_r[:], rhs=sq1[:],
                                     start=True, stop=False)
                    nc.tensor.matmul(ps_s[:], lhsT=ind32_r[:], rhs=sq2[:],
                                     start=False, stop=True)
                    # invc = rsqrt(ms/64 + eps) on the vector engines
                    # (scalar Rsqrt is gated off): magic seed + 2 Newton steps
                    # invc = rsqrt(ms/64 + eps) via exp(-0.5 ln(.)) on scalar
                    # (the direct Rsqrt table is gated off for accuracy)
                    invc = tr.tile([HPC, TB], bf16, tag="invc" + eng, name=f"invc{eng}{n}")
                    lnm = tr.tile([HPC, TB], f32, tag="lnm" + eng, name=f"lnm{eng}{n}", bufs=1)
                    nc.scalar.activation(lnm[:], ps_s[:], Ln,
                                         bias=eps_t[:], scale=1.0 / 64.0)
                    nc.scalar.activation(invc[:], lnm[:], Exp, scale=-0.5)
                    ps_b = ps.tile([P, TB], f32, tag=("pC" if eng == "q" else "pD"),
                                   name=f"psb{eng}{n}")
                    nc.tensor.matmul(ps_b[:], lhsT=bc32_r[:], rhs=invc[:],
                                     start=True, stop=True)
                    nc.vector.tensor_mul(x1[:], x1[:], ps_b[:])
                    nc.vector.tensor_mul(x2[:], x2[:], ps_b[:])
                    # rope -> persistent bf16 tiles
                    m_a = tr.tile([P, TB], f32, tag=ta, name=f"ma{eng}{n}", bufs=1)
                    m_b = tr.tile([P, TB], f32, tag=tb, name=f"mb{eng}{n}", bufs=1)
                    e.tensor_mul(m_a[:], x1[:], cosr_t[:, nsl])
                    e.tensor_mul(m_b[:], x2[:], sinr_t[:, nsl])
                    e.tensor_add(d1[n][:], m_a[:], m_b[:])
                    m_c = tr.tile([P, TB], f32, tag=ta, name=f"mc{eng}{n}", bufs=1)
                    m_d = tr.tile([P, TB], f32, tag=tb, name=f"md{eng}{n}", bufs=1)
                    e.tensor_mul(m_c[:], x2[:], cosr_t[:, nsl])
                    e.tensor_mul(m_d[:], x1[:], sinr_t[:, nsl])
                    e.tensor_sub(d2[n][:], m_c[:], m_d[:])

            # ====== Phase 2: attention + out-projection, per query block ===
            P2_SKIP = bool(_CACHE.get("p2_skip"))
            for j in range(0 if not P2_SKIP else NB, NB):
                jsl = slice(j * TB, (j + 1) * TB)
                n_k = 4 * j + 4
                Y = [ps.tile([65, TB], f32, tag=PT[h], name=f"Y{h}_{j}")
                     for h in range(HPC)]
                # software-pipelined: issue S(k) matmuls one chunk ahead of
                # exp/Y so the tensor engine never sits behind the scalar exp
                S_tiles = [None] * n_k

                def issue_S(k):
                    r = k - 4 * j
                    mtrim = 128 * r if r > 0 else 0
                    msl = slice(mtrim, TB)
                    nb_, kc = k // 4, k % 4
                    ksl = slice(128 * kc, 128 * kc + 128)
                    S_A = ps.tile([P, 2, TB], f32, tag="SA", name=f"SA{j}_{k}")
                    S_B = ps.tile([P, 2, TB], f32, tag="SB", name=f"SB{j}_{k}")
                    for h in range(HPC):
                        X = S_A if h < 2 else S_B
                        hh = h % 2
                        hsl = slice(32 * h, 32 * h + 32)
                        nc.tensor.matmul(X[:, hh, msl], lhsT=k1[nb_][hsl, ksl],
                                         rhs=q1[j][hsl, msl], start=True, stop=False,
                                         tile_position=(32 * h, 0))
                        nc.tensor.matmul(X[:, hh, msl], lhsT=k2[nb_][hsl, ksl],
                                         rhs=q2[j][hsl, msl], start=False, stop=True,
                                         tile_position=(32 * h, 0))
                    S_tiles[k] = (S_A, S_B)

                issue_S(0)
                for k in range(n_k):
                    r = k - 4 * j
                    mtrim = 128 * r if r > 0 else 0
                    msl = slice(mtrim, TB)
                    S_A, S_B = S_tiles[k]
                    e_A = tr.tile([P, 2, TB], bf16, tag="eA", name=f"eA{j}_{k}", bufs=2)
                    e_B = tr.tile([P, 2, TB], bf16, tag="eB", name=f"eB{j}_{k}", bufs=2)
                    for hh in range(2):
                        nc.scalar.activation(e_A[:, hh, msl], S_A[:, hh, msl], Exp, scale=0.125)
                        nc.scalar.activation(e_B[:, hh, msl], S_B[:, hh, msl], Exp, scale=0.125)
                    if r >= 0:
                        tsl = slice(128 * r, 128 * r + 128)
                        for hh in range(2):
                            nc.gpsimd.tensor_mul(e_A[:, hh, tsl], e_A[:, hh, tsl],
                                                 mask_r[:, 0, :])
                            nc.gpsimd.tensor_mul(e_B[:, hh, tsl], e_B[:, hh, tsl],
                                                 mask_r[:, 0, :])
                    if k + 1 < n_k:
                        issue_S(k + 1)
                    st, sp = (k == 0), (k == n_k - 1)
                    for h in range(HPC):
                        e_X = e_A if h < 2 else e_B
                        nc.tensor.matmul(Y[h][:, msl], lhsT=v_r[k][:, h, :],
                                         rhs=e_X[:, h % 2, msl], start=st, stop=sp)
                # copy out: per-head y rows + den row (DMA shifts partitions)
                for h in range(HPC):
                    yb = tr.tile([65, TB], f32, tag="cpbuf", name=f"yb{h}_{j}",
                                 bufs=4, padded_shape=[P, TB])
                    nc.vector.tensor_copy(yb[:], Y[h][:])
                    c, a = h // 2, h % 2
                    nc.sync.dma_start(out=yT32[c][64 * a:64 * a + 64, jsl],
                                      in_=yb[0:64, :])
                    nc.sync.dma_start(out=den4[h:h + 1, jsl], in_=yb[64:65, :])
                # normalize: yT_bf = yT32 * (1/den) broadcast over 64-row groups
                nc.vector.reciprocal(invden4[:, jsl], den4[:, jsl])
                nc.vector.tensor_copy(invden_bf[:, jsl], invden4[:, jsl])
                for c in range(2):
                    ps_i = ps.tile([P, TB], f32, tag=PT[c], name=f"psi{c}{j}")
                    nc.tensor.matmul(ps_i[:], lhsT=selpair_r[:, c * P:(c + 1) * P],
                                     rhs=invden_bf[:, jsl], start=True, stop=True)
                    nc.vector.tensor_mul(yT_bf[c][:, jsl], yT32[c][:, jsl], ps_i[:])
                # out-projection for this query block
                for o in range(8):
                    osl = slice(o * P, (o + 1) * P)
                    po = ps.tile([P, TB], f32, tag=PT[(o % 2) + 2], name=f"po{o}_{j}")
                    nc.tensor.matmul(po[:], lhsT=wp_r[0][:, osl], rhs=yT_bf[0][:, jsl],
                                     start=True, stop=False)
                    nc.tensor.matmul(po[:], lhsT=wp_r[1][:, osl], rhs=yT_bf[1][:, jsl],
                                     start=False, stop=True)
                    ob = tr.tile([P, TB], bf16, tag="ob", name=f"ob{o}_{j}", bufs=3)
                    nc.vector.tensor_copy(ob[:], po[:])
                    nc.sync.dma_start(out=out_d[osl, jsl], in_=ob[:])

            if DEBUG:
                if not bool(_CACHE.get("no_rmsrope")):
                    nc.sync.dma_start(out=dbg_q1[:, :], in_=q1[0][:])
                    nc.sync.dma_start(out=dbg_k1[:, :], in_=k1[0][:])
                if not bool(_CACHE.get("no_v")):
                    nc.sync.dma_start(out=dbg_v[:, :, :], in_=v_r[0][:])
                if not P2_SKIP:
                    nc.sync.dma_start(out=dbg_y[:, :], in_=yT32[0][:])
                    nc.sync.dma_start(out=dbg_den[:, :], in_=den4[:])
                    nc.sync.dma_start(out=dbg_inv[:, :], in_=invden4[:])

    nc.compile()
    return nc


def _get_module():
    if "nc" not in _CACHE:
        _CACHE["nc"] = _build_module()
        _CACHE["consts"] = _build_consts()
    return _CACHE["nc"], _CACHE["consts"]


def _bf16(a):
    return np.ascontiguousarray(a, dtype=np.float32).astype(ml_dtypes.bfloat16)


def _core_inputs(x, w_q, w_k, w_v, w_proj, core):
    """Build the per-core input map (numpy, host-side sharding)."""
    b = core // 4
    g = core % 4
    heads = [4 * g + j for j in range(HPC)]

    xt = _bf16(np.ascontiguousarray(x[b].T))              # [C, T]

    perm = np.empty(256, dtype=np.int64)
    for m in range(128):
        perm[m] = 64 * heads[m // 32] + (m % 32)             # x1 half
        perm[128 + m] = 64 * heads[m // 32] + 32 + (m % 32)  # x2 half
    wq = _bf16(np.ascontiguousarray(w_q[perm, :].T))         # [C, 256]
    wk = _bf16(np.ascontiguousarray(w_k[perm, :].T))

    vperm = np.empty(256, dtype=np.int64)
    for m in range(256):
        vperm[m] = 64 * heads[m // 64] + (m % 64)
    wv = _bf16(np.ascontiguousarray(w_v[vperm, :].T))        # [C, 256]
    wp = _bf16(np.ascontiguousarray(w_proj[:, vperm].T))     # [256, C]

    return dict(xt=xt, wq=wq, wk=wk, wv=wv, wp=wp)


def kernel(x, w_q, w_k, w_v, w_proj, _trace=False, _trace_cores=None):
    from concourse.bass_utils import run_bass_kernel_spmd

    nc, consts = _get_module()
    x = np.asarray(x, dtype=np.float32)
    in_maps = []
    for core in range(N_CORES):
        m = _core_inputs(np.asarray(x), np.asarray(w_q), np.asarray(w_k),
                         np.asarray(w_v), np.asarray(w_proj), core)
        m.update(consts)
        in_maps.append(m)

    res = run_bass_kernel_spmd(nc, in_maps, list(range(N_CORES)),
                               trace=_trace, trace_cores=_trace_cores)
    outs = [np.asarray(res.results[c]["outT"], dtype=np.float32)
            for c in range(N_CORES)]
    out = np.empty((B, T, C), dtype=np.float32)
    for b in range(B):
        acc = outs[4 * b]
        for g in range(1, 4):
            acc = acc + outs[4 * b + g]
        out[b] = acc.T
    if _trace:
        kernel._last_exec_time_ns = res.exec_time_ns
        kernel._last_results = res
    return out
